# revision 1
# baseline (speedup 1.0000x reference)
"""Low-rank ray tracer CSI kernel for 8 Trainium2 NeuronCores.

Reference computation:
    A = einsum('dpr,kr->dk', ua, F); B = einsum('dpr,kr->dk', ub, F)
    csi[k] = sum_d A[d,k]*B[d,k] / D

Since F has no p index, A = (sum_p ua) @ F^T.  Let Ua[d,r] = sum_p ua[d,p,r]
(same for Ub).  Then
    csi[k] = (1/D) * sum_d (Ua F^T)[d,k] (Ub F^T)[d,k]
           = (1/D) * f_k^T (Ua^T Ub) f_k  =  (1/D) * f_k^T M f_k
with M = Ua^T Ub a tiny [R,R] Gram matrix.  Sharding d across cores makes M
additive, and csi is linear in M, so each core returns its partial csi and the
host sums 8 vectors of 4 KB.  The kernel is then purely DMA-bound: each core
streams its 16 MiB shard once; the only non-trivial compute is the p-reduction
on the vector engine, which hides under the DMA.

The host pre-transposes the inputs to [D, R, P] so that the p axis is
contiguous in SBUF: the vector-engine reduce then runs with a stride-1 inner
axis (single-src perf mode) instead of the 4x-slower strided form, and each
chunk reduce writes its Ua columns directly (no second reduction stage).
"""

import sys

import numpy as np

sys.path.insert(0, "/opt/trn_rl_repo")

import concourse.bacc as bacc
import concourse.bass as bass
import concourse.mybir as mybir
from concourse.bass_utils import run_bass_kernel_spmd
from concourse.masks import make_identity
from concourse.tile import TileContext

D, P, R, K = 1024, 256, 64, 1024
NCORES = 8
DC = D // NCORES  # directions per core
RC = 16  # r-chunk per DMA/reduce step (input layout [D, R, P])
NCH = R // RC  # chunks per tensor
KC = K // 128  # k chunks of 128 (PSUM partition limit)

F32 = mybir.dt.float32


def build_bass() -> bass.Bass:
    nc = bacc.Bacc(None, target_bir_lowering=False)
    # per-core shards, pre-transposed to [d, r, p]
    ua = nc.declare_dram_parameter("ua", [DC, R, P], F32, isOutput=False)
    ub = nc.declare_dram_parameter("ub", [DC, R, P], F32, isOutput=False)
    f = nc.declare_dram_parameter("f", [K, R], F32, isOutput=False)
    # out[p, c] = partial csi[c*128 + p], already scaled by 1/D
    out = nc.declare_dram_parameter("out", [128, KC], F32, isOutput=True)

    with TileContext(nc) as tc:
        with (
            tc.tile_pool(name="const", bufs=1) as cpool,
            tc.tile_pool(name="chunks", bufs=2 * NCH) as chpool,
            tc.tile_pool(name="small", bufs=1) as spool,
            tc.tile_pool(name="scratch", bufs=2) as scpool,
            tc.tile_pool(name="psum", bufs=2, space="PSUM") as ppool,
            tc.tile_pool(name="psum1", bufs=1, space="PSUM") as ppool1,
        ):
            identity = cpool.tile([128, 128], F32)
            make_identity(nc, identity[:])

            # F in natural layout, k on partitions: [128, KC, R]
            f_sb = cpool.tile([128, KC, R], F32)
            nc.sync.dma_start(out=f_sb[:], in_=f.rearrange("(c p) r -> p c r", p=128))

            # F^T [R, K] via PE transposes of the natural chunks
            ft_sb = cpool.tile([R, K], F32)
            for c in range(KC):
                ftp = ppool.tile([R, 128], F32, tag="ftp")
                nc.tensor.transpose(ftp[:], f_sb[:, c, :], identity[:])
                nc.vector.tensor_copy(out=ft_sb[:, c * 128 : (c + 1) * 128], in_=ftp[:])

            # Streaming p-reduction: Ua[d,r] = sum_p ua[d,r,p] (same for ub)
            us = []
            for name, t_ap in (("a", ua), ("b", ub)):
                u = spool.tile([DC, R], F32, tag=f"u_{name}")
                for i in range(NCH):
                    ch = chpool.tile([DC, RC, P], F32, tag="chunk")
                    nc.sync.dma_start(out=ch[:], in_=t_ap[:, i * RC : (i + 1) * RC, :])
                    nc.vector.tensor_reduce(
                        out=u[:, i * RC : (i + 1) * RC],
                        in_=ch[:],
                        axis=mybir.AxisListType.X,
                        op=mybir.AluOpType.add,
                    )
                us.append(u)

            # Gram matrix M[r1,r2] = sum_d Ua[d,r1] Ub[d,r2]
            m_psum = ppool1.tile([R, R], F32)
            nc.tensor.matmul(m_psum[:], us[0][:], us[1][:], start=True, stop=True)
            # fold the 1/D normalization into M while copying out of PSUM
            m_sb = spool.tile([R, R], F32)
            nc.scalar.mul(m_sb[:], m_psum[:], 1.0 / D)

            # csi[k] = sum_r2 (sum_r1 F[k,r1] (M/D)[r1,r2]) * F[k,r2]
            csi = spool.tile([128, KC], F32)
            for c in range(KC):
                g_psum = ppool.tile([128, R], F32, tag="g")
                nc.tensor.matmul(
                    g_psum[:],
                    ft_sb[:, c * 128 : (c + 1) * 128],
                    m_sb[:],
                    start=True,
                    stop=True,
                )
                scr = scpool.tile([128, R], F32, tag="scr")
                nc.vector.tensor_mul(out=scr[:], in0=g_psum[:], in1=f_sb[:, c, :])
                nc.vector.tensor_reduce(
                    out=csi[:, c : c + 1],
                    in_=scr[:],
                    axis=mybir.AxisListType.X,
                    op=mybir.AluOpType.add,
                )
            nc.sync.dma_start(out=out[:], in_=csi[:])
    nc.compile()
    return nc


_NC_CACHE = None


def kernel(**inputs: np.ndarray) -> np.ndarray:
    global _NC_CACHE
    ua = np.asarray(inputs["attenuation_vectors"], dtype=np.float32)
    ub = np.asarray(inputs["radiation_vectors"], dtype=np.float32)
    f = np.ascontiguousarray(inputs["frequency_basis_vectors"], dtype=np.float32)

    # [D, P, R] -> [D, R, P] so the p axis is contiguous on-device
    ua_t = np.ascontiguousarray(ua.transpose(0, 2, 1))
    ub_t = np.ascontiguousarray(ub.transpose(0, 2, 1))

    if _NC_CACHE is None:
        _NC_CACHE = build_bass()
    nc = _NC_CACHE

    in_maps = [
        {
            "ua": ua_t[c * DC : (c + 1) * DC],
            "ub": ub_t[c * DC : (c + 1) * DC],
            "f": f,
        }
        for c in range(NCORES)
    ]
    res = run_bass_kernel_spmd(nc, in_maps, list(range(NCORES)))
    acc = np.zeros((128, KC), dtype=np.float32)
    for r in res.results:
        acc += r["out"]
    return acc.T.reshape(K).astype(np.float32)


if __name__ == "__main__":
    rng = np.random.default_rng(0)
    ins = {
        "attenuation_vectors": rng.standard_normal((D, P, R), dtype=np.float32),
        "radiation_vectors": rng.standard_normal((D, P, R), dtype=np.float32),
        "frequency_basis_vectors": rng.standard_normal((K, R), dtype=np.float32),
    }
    got = kernel(**ins)
    ua_s = ins["attenuation_vectors"].sum(axis=1)
    ub_s = ins["radiation_vectors"].sum(axis=1)
    a = ua_s @ ins["frequency_basis_vectors"].T
    b = ub_s @ ins["frequency_basis_vectors"].T
    want = (a * b).sum(axis=0) / D
    err = np.abs(got - want).max() / np.abs(want).max()
    print("rel err vs local numpy:", err)



# revision 5
# speedup vs baseline: 2.0433x; 2.0433x over previous
"""Low-rank ray tracer CSI kernel for 8 Trainium2 NeuronCores.

Reference computation:
    A = einsum('dpr,kr->dk', ua, F); B = einsum('dpr,kr->dk', ub, F)
    csi[k] = sum_d A[d,k]*B[d,k] / D

Since F has no p index, A = (sum_p ua) @ F^T.  Let Ua[d,r] = sum_p ua[d,p,r]
(same for Ub).  Then
    csi[k] = (1/D) * f_k^T (Ua^T Ub) f_k  =  f'_k^T M f'_k
with M = Ua^T Ub a tiny [R,R] Gram matrix and f' = f/sqrt(D) (scaling folded
into F on the host).  Sharding d across cores makes M additive and csi linear
in M, so each core returns its partial csi and the host sums 8 vectors of 4KB.

The kernel is DMA-bound: each core streams its shard once.  To halve HBM
traffic the host casts the inputs to fp16 (the 2e-2 rel-err budget dwarfs
fp16's ~5e-4).  The p-reduction is split between engines so neither blocks
the DMA stream: the host pre-splits p into S=4 slices (layout [D, S, R, P/S]),
the otherwise-idle PE folds the slices with accumulating identity matmuls
(PSUM += I @ slice), and the DVE only reduces the remaining P/S=64-wide axis.
"""

import sys

import numpy as np

sys.path.insert(0, "/opt/trn_rl_repo")

import concourse.bacc as bacc
import concourse.bass as bass
import concourse.mybir as mybir
from concourse.bass_utils import run_bass_kernel_spmd
from concourse.masks import make_identity
from concourse.tile import TileContext

D, P, R, K = 1024, 256, 64, 1024
NCORES = 8
DC = D // NCORES  # directions per core
S = 4  # p-slices folded on the PE
PS = P // S  # p per slice after the fold
RC = 16  # r-chunk per DMA/fold/reduce step
NCH = R // RC  # chunks per tensor
KC = K // 128  # k chunks of 128 (PSUM partition limit)
MM_N = 512  # max matmul free size (one PSUM bank)

F32 = mybir.dt.float32
F16 = mybir.dt.float16


def build_bass() -> bass.Bass:
    nc = bacc.Bacc(None, target_bir_lowering=False)
    # per-core shards, fp16, p split into S slices: [d, s, r, p/S]
    ua = nc.declare_dram_parameter("ua", [DC, S, R, PS], F16, isOutput=False)
    ub = nc.declare_dram_parameter("ub", [DC, S, R, PS], F16, isOutput=False)
    # F/sqrt(D) with k on partitions: [128, KC, R] fp32
    f = nc.declare_dram_parameter("f", [128, KC, R], F32, isOutput=False)
    # (F/sqrt(D))^T: [R, K] fp16 (matmul lhsT)
    ft = nc.declare_dram_parameter("ft", [R, K], F16, isOutput=False)
    # out[p, c] = partial csi[c*128 + p]
    out = nc.declare_dram_parameter("out", [128, KC], F32, isOutput=True)

    with TileContext(nc) as tc:
        with (
            tc.tile_pool(name="const", bufs=1) as cpool,
            tc.tile_pool(name="chunks", bufs=2 * NCH) as chpool,
            tc.tile_pool(name="small", bufs=1) as spool,
            tc.tile_pool(name="scratch", bufs=2) as scpool,
            tc.tile_pool(name="pfold", bufs=2, space="PSUM") as fpool,
            tc.tile_pool(name="pm", bufs=1, space="PSUM") as mpool,
            tc.tile_pool(name="pg", bufs=2, space="PSUM") as gpool,
        ):
            identity = cpool.tile([128, 128], F16)
            make_identity(nc, identity[:])

            # Streaming p-reduction: Ua[d,r] = sum_{s,p} ua[d,s,r,p]
            us = []
            for name, t_ap in (("a", ua), ("b", ub)):
                u = spool.tile([DC, R], F16, tag=f"u_{name}")
                us.append(u)
            for i in range(NCH):
                for t_ap, u in ((ua, us[0]), (ub, us[1])):
                    ch = chpool.tile([DC, S, RC, PS], F16, tag="chunk")
                    nc.sync.dma_start(out=ch[:], in_=t_ap[:, :, i * RC : (i + 1) * RC, :])
                    # PE fold over s: psum[d, rc, p] = sum_s ch[d, s, rc, p]
                    pf = fpool.tile([DC, RC, PS], F32, tag="fold")
                    ncol = RC * PS
                    nh = ncol // MM_N
                    for h in range(nh):
                        r0 = h * (MM_N // PS)
                        r1 = (h + 1) * (MM_N // PS)
                        for s in range(S):
                            nc.tensor.matmul(
                                pf[:, r0:r1, :],
                                identity[:],
                                ch[:, s, r0:r1, :],
                                start=(s == 0),
                                stop=(s == S - 1),
                            )
                    # DVE reduce over the remaining p axis
                    with nc.allow_low_precision(reason="fp16 path is within tolerance"):
                        nc.vector.tensor_reduce(
                            out=u[:, i * RC : (i + 1) * RC],
                            in_=pf[:],
                            axis=mybir.AxisListType.X,
                            op=mybir.AluOpType.add,
                        )

            # F tiles (queued after the bulk stream; needed only for the tail)
            f_sb = cpool.tile([128, KC, R], F32)
            nc.sync.dma_start(out=f_sb[:], in_=f[:])
            ft_sb = cpool.tile([R, K], F16)
            nc.sync.dma_start(out=ft_sb[:], in_=ft[:])

            # Gram matrix M[r1,r2] = sum_d Ua[d,r1] Ub[d,r2]
            m_psum = mpool.tile([R, R], F32, tag="m")
            nc.tensor.matmul(m_psum[:], us[0][:], us[1][:], start=True, stop=True)
            m_sb = spool.tile([R, R], F16)
            with nc.allow_low_precision(reason="fp16 M is within tolerance"):
                nc.vector.tensor_copy(out=m_sb[:], in_=m_psum[:])

            # csi[k] = sum_r2 (sum_r1 F'[k,r1] M[r1,r2]) * F'[k,r2]
            csi = spool.tile([128, KC], F32)
            for c in range(KC):
                g_psum = gpool.tile([128, R], F32, tag="g")
                nc.tensor.matmul(
                    g_psum[:],
                    ft_sb[:, c * 128 : (c + 1) * 128],
                    m_sb[:],
                    start=True,
                    stop=True,
                )
                scr = scpool.tile([128, R], F32, tag="scr")
                nc.vector.tensor_mul(out=scr[:], in0=g_psum[:], in1=f_sb[:, c, :])
                nc.vector.tensor_reduce(
                    out=csi[:, c : c + 1],
                    in_=scr[:],
                    axis=mybir.AxisListType.X,
                    op=mybir.AluOpType.add,
                )
            nc.sync.dma_start(out=out[:], in_=csi[:])
    nc.compile()
    return nc


def _prep_inputs(ua, ub, f):
    """Host-side layout prep shared by kernel() and test harnesses."""
    # [D, P, R] fp32 -> [D, S, R, P/S] fp16 (p split outer for the PE fold)
    ua16 = ua.astype(np.float16).reshape(D, S, PS, R).transpose(0, 1, 3, 2)
    ub16 = ub.astype(np.float16).reshape(D, S, PS, R).transpose(0, 1, 3, 2)
    ua16 = np.ascontiguousarray(ua16)
    ub16 = np.ascontiguousarray(ub16)
    fs = (f / np.sqrt(np.float32(D))).astype(np.float32)
    f_host = np.ascontiguousarray(fs.reshape(KC, 128, R).transpose(1, 0, 2))
    ft_host = np.ascontiguousarray(fs.T.astype(np.float16))
    return ua16, ub16, f_host, ft_host


_NC_CACHE = None


def kernel(**inputs: np.ndarray) -> np.ndarray:
    global _NC_CACHE
    ua = np.asarray(inputs["attenuation_vectors"], dtype=np.float32)
    ub = np.asarray(inputs["radiation_vectors"], dtype=np.float32)
    f = np.asarray(inputs["frequency_basis_vectors"], dtype=np.float32)

    ua16, ub16, f_host, ft_host = _prep_inputs(ua, ub, f)

    if _NC_CACHE is None:
        _NC_CACHE = build_bass()
    nc = _NC_CACHE

    in_maps = [
        {
            "ua": ua16[c * DC : (c + 1) * DC],
            "ub": ub16[c * DC : (c + 1) * DC],
            "f": f_host,
            "ft": ft_host,
        }
        for c in range(NCORES)
    ]
    res = run_bass_kernel_spmd(nc, in_maps, list(range(NCORES)))
    acc = np.zeros((128, KC), dtype=np.float32)
    for r in res.results:
        acc += r["out"]
    return acc.T.reshape(K).astype(np.float32)


if __name__ == "__main__":
    rng = np.random.default_rng(0)
    ins = {
        "attenuation_vectors": rng.standard_normal((D, P, R), dtype=np.float32),
        "radiation_vectors": rng.standard_normal((D, P, R), dtype=np.float32),
        "frequency_basis_vectors": rng.standard_normal((K, R), dtype=np.float32),
    }
    got = kernel(**ins)
    ua_s = ins["attenuation_vectors"].sum(axis=1)
    ub_s = ins["radiation_vectors"].sum(axis=1)
    a = ua_s @ ins["frequency_basis_vectors"].T
    b = ub_s @ ins["frequency_basis_vectors"].T
    want = (a * b).sum(axis=0) / D
    err = np.abs(got - want).max() / np.abs(want).max()
    print("rel err vs local numpy:", err)


# revision 6
# speedup vs baseline: 2.3202x; 1.1355x over previous
"""Low-rank ray tracer CSI kernel for 8 Trainium2 NeuronCores.

Reference computation:
    A = einsum('dpr,kr->dk', ua, F); B = einsum('dpr,kr->dk', ub, F)
    csi[k] = sum_d A[d,k]*B[d,k] / D

Since F has no p index, A = (sum_p ua) @ F^T.  Let Ua[d,r] = sum_p ua[d,p,r]
(same for Ub).  Then
    csi[k] = (1/D) * f_k^T (Ua^T Ub) f_k  =  f'_k^T M f'_k
with M = Ua^T Ub a tiny [R,R] Gram matrix and f' = f/sqrt(D) (scaling folded
into F on the host).  Sharding d across cores makes M additive and csi linear
in M, so each core returns its partial csi and the host sums 8 vectors of 4KB.

The kernel is DMA-bound: each core streams its shard once.  To halve HBM
traffic the host casts the inputs to fp16 (the 2e-2 rel-err budget dwarfs
fp16's ~5e-4).  The p-reduction is split between engines so neither blocks
the DMA stream: the host pre-splits p into S=4 slices (layout [D, S, R, P/S]),
the otherwise-idle PE folds the slices with accumulating identity matmuls
(PSUM += I @ slice), and the DVE only reduces the remaining P/S=64-wide axis.
The r-chunks are tapered (16,16,16,12,4) so the final fold+reduce on the
critical path after the last DMA is small.
"""

import sys

import numpy as np

sys.path.insert(0, "/opt/trn_rl_repo")

import concourse.bacc as bacc
import concourse.bass as bass
import concourse.mybir as mybir
from concourse.bass_utils import run_bass_kernel_spmd
from concourse.masks import make_identity
from concourse.tile import TileContext

D, P, R, K = 1024, 256, 64, 1024
NCORES = 8
DC = D // NCORES  # directions per core
S = 4  # p-slices folded on the PE
PS = P // S  # p per slice after the fold
CHUNKS = (16, 16, 16, 12, 4)  # r-chunk sizes (sum = R); tapered for the tail
KC = K // 128  # k chunks of 128 (PSUM partition limit)
WIN = 512 // PS  # r-rows per PSUM-bank-sized matmul window

F32 = mybir.dt.float32
F16 = mybir.dt.float16


def build_bass() -> bass.Bass:
    nc = bacc.Bacc(None, target_bir_lowering=False)
    # per-core shards, fp16, p split into S slices: [d, s, r, p/S]
    ua = nc.declare_dram_parameter("ua", [DC, S, R, PS], F16, isOutput=False)
    ub = nc.declare_dram_parameter("ub", [DC, S, R, PS], F16, isOutput=False)
    # F/sqrt(D) with k on partitions: [128, KC, R] fp16
    f = nc.declare_dram_parameter("f", [128, KC, R], F16, isOutput=False)
    # (F/sqrt(D))^T: [R, K] fp16 (matmul lhsT)
    ft = nc.declare_dram_parameter("ft", [R, K], F16, isOutput=False)
    # out[p, c] = partial csi[c*128 + p]
    out = nc.declare_dram_parameter("out", [128, KC], F32, isOutput=True)

    with TileContext(nc) as tc:
        with (
            tc.tile_pool(name="const", bufs=1) as cpool,
            tc.tile_pool(name="chunks", bufs=6) as chpool,
            tc.tile_pool(name="small", bufs=1) as spool,
            tc.tile_pool(name="scratch", bufs=1) as scpool,
            tc.tile_pool(name="pfold", bufs=2, space="PSUM") as fpool,
            tc.tile_pool(name="pm", bufs=1, space="PSUM") as mpool,
            tc.tile_pool(name="pg", bufs=1, space="PSUM") as gpool,
        ):
            identity = cpool.tile([128, 128], F16)
            make_identity(nc, identity[:])

            u_a = spool.tile([DC, R], F16, tag="u_a")
            u_b = spool.tile([DC, R], F16, tag="u_b")

            with nc.allow_low_precision(reason="fp16 path is within tolerance"):
                # Streaming p-reduction: Ua[d,r] = sum_{s,p} ua[d,s,r,p]
                base = 0
                for rc in CHUNKS:
                    for t_ap, u in ((ua, u_a), (ub, u_b)):
                        ch = chpool.tile([DC, S, rc, PS], F16, tag="chunk")
                        nc.sync.dma_start(
                            out=ch[:], in_=t_ap[:, :, base : base + rc, :]
                        )
                        # PE fold over s: pf[d, rc, p] = sum_s ch[d, s, rc, p],
                        # one PSUM-bank window at a time; DVE reduces each
                        # window over p as soon as its fold completes.
                        pf = fpool.tile([DC, rc, PS], F32, tag="fold")
                        for w0 in range(0, rc, WIN):
                            w1 = min(w0 + WIN, rc)
                            for s in range(S):
                                nc.tensor.matmul(
                                    pf[:, w0:w1, :],
                                    identity[:],
                                    ch[:, s, w0:w1, :],
                                    start=(s == 0),
                                    stop=(s == S - 1),
                                )
                            nc.vector.tensor_reduce(
                                out=u[:, base + w0 : base + w1],
                                in_=pf[:, w0:w1, :],
                                axis=mybir.AxisListType.X,
                                op=mybir.AluOpType.add,
                            )
                    base += rc

                # F tiles (queued after the bulk stream; needed only at tail)
                f_sb = cpool.tile([128, KC, R], F16)
                nc.sync.dma_start(out=f_sb[:], in_=f[:])
                ft_sb = cpool.tile([R, K], F16)
                nc.sync.dma_start(out=ft_sb[:], in_=ft[:])

                # Gram matrix M[r1,r2] = sum_d Ua[d,r1] Ub[d,r2]
                m_psum = mpool.tile([R, R], F32, tag="m")
                nc.tensor.matmul(m_psum[:], u_a[:], u_b[:], start=True, stop=True)
                m_sb = spool.tile([R, R], F16, tag="m_sb")
                nc.vector.tensor_copy(out=m_sb[:], in_=m_psum[:])

                # g[k, c, r2] = sum_r1 F'[k+128c, r1] M[r1, r2], all KC chunks
                # into ONE PSUM tile so the csi tail is one mul + one reduce.
                g_psum = gpool.tile([128, KC, R], F32, tag="g")
                for c in range(KC):
                    nc.tensor.matmul(
                        g_psum[:, c, :],
                        ft_sb[:, c * 128 : (c + 1) * 128],
                        m_sb[:],
                        start=True,
                        stop=True,
                    )
                scr = scpool.tile([128, KC, R], F32, tag="scr")
                nc.vector.tensor_mul(out=scr[:], in0=g_psum[:], in1=f_sb[:])
                csi = spool.tile([128, KC], F32, tag="csi")
                nc.vector.tensor_reduce(
                    out=csi[:],
                    in_=scr[:],
                    axis=mybir.AxisListType.X,
                    op=mybir.AluOpType.add,
                )
            nc.sync.dma_start(out=out[:], in_=csi[:])
    nc.compile()
    return nc


def _prep_inputs(ua, ub, f):
    """Host-side layout prep shared by kernel() and test harnesses."""
    # [D, P, R] fp32 -> [D, S, R, P/S] fp16 (p split outer for the PE fold)
    ua16 = ua.astype(np.float16).reshape(D, S, PS, R).transpose(0, 1, 3, 2)
    ub16 = ub.astype(np.float16).reshape(D, S, PS, R).transpose(0, 1, 3, 2)
    ua16 = np.ascontiguousarray(ua16)
    ub16 = np.ascontiguousarray(ub16)
    fs = (f / np.sqrt(np.float32(D))).astype(np.float32)
    f_host = np.ascontiguousarray(
        fs.reshape(KC, 128, R).transpose(1, 0, 2).astype(np.float16)
    )
    ft_host = np.ascontiguousarray(fs.T.astype(np.float16))
    return ua16, ub16, f_host, ft_host


_NC_CACHE = None


def kernel(**inputs: np.ndarray) -> np.ndarray:
    global _NC_CACHE
    ua = np.asarray(inputs["attenuation_vectors"], dtype=np.float32)
    ub = np.asarray(inputs["radiation_vectors"], dtype=np.float32)
    f = np.asarray(inputs["frequency_basis_vectors"], dtype=np.float32)

    ua16, ub16, f_host, ft_host = _prep_inputs(ua, ub, f)

    if _NC_CACHE is None:
        _NC_CACHE = build_bass()
    nc = _NC_CACHE

    in_maps = [
        {
            "ua": ua16[c * DC : (c + 1) * DC],
            "ub": ub16[c * DC : (c + 1) * DC],
            "f": f_host,
            "ft": ft_host,
        }
        for c in range(NCORES)
    ]
    res = run_bass_kernel_spmd(nc, in_maps, list(range(NCORES)))
    acc = np.zeros((128, KC), dtype=np.float32)
    for r in res.results:
        acc += r["out"]
    return acc.T.reshape(K).astype(np.float32)


if __name__ == "__main__":
    rng = np.random.default_rng(0)
    ins = {
        "attenuation_vectors": rng.standard_normal((D, P, R), dtype=np.float32),
        "radiation_vectors": rng.standard_normal((D, P, R), dtype=np.float32),
        "frequency_basis_vectors": rng.standard_normal((K, R), dtype=np.float32),
    }
    got = kernel(**ins)
    ua_s = ins["attenuation_vectors"].sum(axis=1)
    ub_s = ins["radiation_vectors"].sum(axis=1)
    a = ua_s @ ins["frequency_basis_vectors"].T
    b = ub_s @ ins["frequency_basis_vectors"].T
    want = (a * b).sum(axis=0) / D
    err = np.abs(got - want).max() / np.abs(want).max()
    print("rel err vs local numpy:", err)


# revision 9
# speedup vs baseline: 2.7007x; 1.1640x over previous
"""Low-rank ray tracer CSI kernel for 8 Trainium2 NeuronCores.

Reference computation:
    A = einsum('dpr,kr->dk', ua, F); B = einsum('dpr,kr->dk', ub, F)
    csi[k] = sum_d A[d,k]*B[d,k] / D

Since F has no p index, A = (sum_p ua) @ F^T.  Let Ua[d,r] = sum_p ua[d,p,r]
(same for Ub).  Then
    csi[k] = (1/D) * f_k^T (Ua^T Ub) f_k  =  f'_k^T M f'_k
with M = Ua^T Ub a tiny [R,R] Gram matrix and f' = f/sqrt(D) (scaling folded
into F on the host).  Sharding d across cores makes M additive and csi linear
in M, so each core returns its partial csi and the host sums 8 vectors of 4KB.

The kernel is DMA-bound: each core streams its shard once.  To halve HBM
traffic the host casts the inputs to fp16 (the 2e-2 rel-err budget dwarfs
fp16's ~5e-4).  The p-reduction is split between engines so neither blocks
the DMA stream: the host pre-splits p into S=4 slices (layout [D, S, R, P/S]),
the otherwise-idle PE folds the slices with accumulating identity matmuls
(PSUM += I @ slice), and the DVE only reduces the remaining P/S=64-wide axis.
The r-chunks are tapered (16,16,16,12,4) so the final fold+reduce on the
critical path after the last DMA is small.
"""

import sys

import numpy as np

sys.path.insert(0, "/opt/trn_rl_repo")

import concourse.bacc as bacc
import concourse.bass as bass
import concourse.mybir as mybir
from concourse.bass_utils import run_bass_kernel_spmd
from concourse.masks import make_identity
from concourse.tile import TileContext

D, P, R, K = 1024, 256, 64, 1024
NCORES = 8
DC = D // NCORES  # directions per core
S = 4  # p-slices folded on the PE
PS = P // S  # p per slice after the fold
CHUNKS = (16, 16, 16, 12, 4)  # r-chunk sizes (sum = R); tapered for the tail
KC = K // 128  # k chunks of 128 (PSUM partition limit)
WIN = 512 // PS  # r-rows per PSUM-bank-sized matmul window
FOLD_RC = 16  # max r-rows per PSUM fold tile (2 banks)
ALT_QUEUES = True  # alternate chunk DMAs between sync and scalar HWDGE queues

F32 = mybir.dt.float32
F16 = mybir.dt.float16


def build_bass() -> bass.Bass:
    nc = bacc.Bacc(None, target_bir_lowering=False)
    # per-core shards, fp16, p split into S slices: [d, s, r, p/S]
    ua = nc.declare_dram_parameter("ua", [DC, S, R, PS], F16, isOutput=False)
    ub = nc.declare_dram_parameter("ub", [DC, S, R, PS], F16, isOutput=False)
    # F/sqrt(D) with k on partitions: [128, KC, R] fp16
    f = nc.declare_dram_parameter("f", [128, KC, R], F16, isOutput=False)
    # (F/sqrt(D))^T: [R, K] fp16 (matmul lhsT)
    ft = nc.declare_dram_parameter("ft", [R, K], F16, isOutput=False)
    # out[p, c] = partial csi[c*128 + p]
    out = nc.declare_dram_parameter("out", [128, KC], F32, isOutput=True)

    with TileContext(nc) as tc:
        with (
            tc.tile_pool(name="const", bufs=1) as cpool,
            tc.tile_pool(name="chunks", bufs=6) as chpool,
            tc.tile_pool(name="small", bufs=1) as spool,
            tc.tile_pool(name="scratch", bufs=1) as scpool,
            tc.tile_pool(name="pfold", bufs=2, space="PSUM") as fpool,
            tc.tile_pool(name="pm", bufs=1, space="PSUM") as mpool,
            tc.tile_pool(name="pg", bufs=1, space="PSUM") as gpool,
        ):
            identity = cpool.tile([128, 128], F16)
            make_identity(nc, identity[:])

            u_a = spool.tile([DC, R], F16, tag="u_a")
            u_b = spool.tile([DC, R], F16, tag="u_b")

            with nc.allow_low_precision(reason="fp16 path is within tolerance"):
                # Streaming p-reduction: Ua[d,r] = sum_{s,p} ua[d,s,r,p]
                base = 0
                qi = 0
                for rc in CHUNKS:
                    for t_ap, u in ((ua, u_a), (ub, u_b)):
                        ch = chpool.tile([DC, S, rc, PS], F16, tag="chunk")
                        eng = nc.scalar if (ALT_QUEUES and qi % 2) else nc.sync
                        eng.dma_start(
                            out=ch[:], in_=t_ap[:, :, base : base + rc, :]
                        )
                        qi += 1
                        # PE fold over s: pf[d, fc, p] = sum_s ch[d, s, fc, p],
                        # one PSUM-bank window at a time; DVE reduces each
                        # window over p as soon as its fold completes.
                        for f0 in range(0, rc, FOLD_RC):
                            f1 = min(f0 + FOLD_RC, rc)
                            pf = fpool.tile([DC, f1 - f0, PS], F32, tag="fold")
                            for w0 in range(0, f1 - f0, WIN):
                                w1 = min(w0 + WIN, f1 - f0)
                                for s in range(S):
                                    nc.tensor.matmul(
                                        pf[:, w0:w1, :],
                                        identity[:],
                                        ch[:, s, f0 + w0 : f0 + w1, :],
                                        start=(s == 0),
                                        stop=(s == S - 1),
                                    )
                                nc.vector.tensor_reduce(
                                    out=u[:, base + f0 + w0 : base + f0 + w1],
                                    in_=pf[:, w0:w1, :],
                                    axis=mybir.AxisListType.X,
                                    op=mybir.AluOpType.add,
                                )
                    base += rc

                # F tiles (queued after the bulk stream; needed only at tail)
                f_sb = cpool.tile([128, KC, R], F16)
                nc.sync.dma_start(out=f_sb[:], in_=f[:])
                ft_sb = cpool.tile([R, K], F16)
                nc.sync.dma_start(out=ft_sb[:], in_=ft[:])

                # Gram matrix M[r1,r2] = sum_d Ua[d,r1] Ub[d,r2]
                m_psum = mpool.tile([R, R], F32, tag="m")
                nc.tensor.matmul(m_psum[:], u_a[:], u_b[:], start=True, stop=True)
                m_sb = spool.tile([R, R], F16, tag="m_sb")
                nc.vector.tensor_copy(out=m_sb[:], in_=m_psum[:])

                # g[k, c, r2] = sum_r1 F'[k+128c, r1] M[r1, r2], all KC chunks
                # into ONE PSUM tile so the csi tail is one mul + one reduce.
                g_psum = gpool.tile([128, KC, R], F32, tag="g")
                for c in range(KC):
                    nc.tensor.matmul(
                        g_psum[:, c, :],
                        ft_sb[:, c * 128 : (c + 1) * 128],
                        m_sb[:],
                        start=True,
                        stop=True,
                    )
                scr = scpool.tile([128, KC, R], F32, tag="scr")
                nc.vector.tensor_mul(out=scr[:], in0=g_psum[:], in1=f_sb[:])
                csi = spool.tile([128, KC], F32, tag="csi")
                nc.vector.tensor_reduce(
                    out=csi[:],
                    in_=scr[:],
                    axis=mybir.AxisListType.X,
                    op=mybir.AluOpType.add,
                )
            nc.sync.dma_start(out=out[:], in_=csi[:])
    nc.compile()
    return nc


def _prep_inputs(ua, ub, f):
    """Host-side layout prep shared by kernel() and test harnesses."""
    # [D, P, R] fp32 -> [D, S, R, P/S] fp16 (p split outer for the PE fold)
    ua16 = ua.astype(np.float16).reshape(D, S, PS, R).transpose(0, 1, 3, 2)
    ub16 = ub.astype(np.float16).reshape(D, S, PS, R).transpose(0, 1, 3, 2)
    ua16 = np.ascontiguousarray(ua16)
    ub16 = np.ascontiguousarray(ub16)
    fs = (f / np.sqrt(np.float32(D))).astype(np.float32)
    f_host = np.ascontiguousarray(
        fs.reshape(KC, 128, R).transpose(1, 0, 2).astype(np.float16)
    )
    ft_host = np.ascontiguousarray(fs.T.astype(np.float16))
    return ua16, ub16, f_host, ft_host


_NC_CACHE = None


def kernel(**inputs: np.ndarray) -> np.ndarray:
    global _NC_CACHE
    ua = np.asarray(inputs["attenuation_vectors"], dtype=np.float32)
    ub = np.asarray(inputs["radiation_vectors"], dtype=np.float32)
    f = np.asarray(inputs["frequency_basis_vectors"], dtype=np.float32)

    ua16, ub16, f_host, ft_host = _prep_inputs(ua, ub, f)

    if _NC_CACHE is None:
        _NC_CACHE = build_bass()
    nc = _NC_CACHE

    in_maps = [
        {
            "ua": ua16[c * DC : (c + 1) * DC],
            "ub": ub16[c * DC : (c + 1) * DC],
            "f": f_host,
            "ft": ft_host,
        }
        for c in range(NCORES)
    ]
    res = run_bass_kernel_spmd(nc, in_maps, list(range(NCORES)))
    acc = np.zeros((128, KC), dtype=np.float32)
    for r in res.results:
        acc += r["out"]
    return acc.T.reshape(K).astype(np.float32)


if __name__ == "__main__":
    rng = np.random.default_rng(0)
    ins = {
        "attenuation_vectors": rng.standard_normal((D, P, R), dtype=np.float32),
        "radiation_vectors": rng.standard_normal((D, P, R), dtype=np.float32),
        "frequency_basis_vectors": rng.standard_normal((K, R), dtype=np.float32),
    }
    got = kernel(**ins)
    ua_s = ins["attenuation_vectors"].sum(axis=1)
    ub_s = ins["radiation_vectors"].sum(axis=1)
    a = ua_s @ ins["frequency_basis_vectors"].T
    b = ub_s @ ins["frequency_basis_vectors"].T
    want = (a * b).sum(axis=0) / D
    err = np.abs(got - want).max() / np.abs(want).max()
    print("rel err vs local numpy:", err)


# revision 21
# speedup vs baseline: 2.9726x; 1.1007x over previous
"""Low-rank ray tracer CSI kernel for 8 Trainium2 NeuronCores.

Reference computation:
    A = einsum('dpr,kr->dk', ua, F); B = einsum('dpr,kr->dk', ub, F)
    csi[k] = sum_d A[d,k]*B[d,k] / D

Since F has no p index, A = (sum_p ua) @ F^T.  Let Ua[d,r] = sum_p ua[d,p,r]
(same for Ub).  Then
    csi[k] = (1/D) * f_k^T (Ua^T Ub) f_k  =  f'_k^T M f'_k
with M = Ua^T Ub a tiny [R,R] Gram matrix and f' = f/sqrt(D) (scaling folded
into F on the host).  Sharding d across cores makes M additive and csi linear
in M, so each core returns its partial csi and the host sums 8 vectors of 4KB.

The kernel is DMA-bound: each core streams its shard once.  To halve HBM
traffic the host casts the inputs to fp16 (the 2e-2 rel-err budget dwarfs
fp16's ~5e-4).  The p-reduction is split between engines so neither blocks
the DMA stream: the host pre-splits p into S=4 slices (layout [D, S, R, P/S]),
the otherwise-idle PE folds the slices with accumulating identity matmuls
(PSUM += I @ slice), and the DVE only reduces the remaining P/S=64-wide axis.
The r-chunks are tapered (8,...,8,6,2) so the final fold+reduce on the
critical path after the last DMA is small; 8-row chunks keep DMA descriptors
at 1KB, which still saturates the ~358 GB/s per-core HBM limit (the SDMA
fabric overhead at 1KB descriptors stays above the HBM ceiling).
"""

import sys

import numpy as np

sys.path.insert(0, "/opt/trn_rl_repo")

import concourse.bacc as bacc
import concourse.bass as bass
import concourse.mybir as mybir
from concourse.bass_utils import run_bass_kernel_spmd
from concourse.masks import make_identity
from concourse.tile import TileContext

D, P, R, K = 1024, 256, 64, 1024
NCORES = 8
DC = D // NCORES  # directions per core
S = 4  # p-slices folded on the PE
PS = P // S  # p per slice after the fold
CHUNKS = (8, 8, 8, 8, 8, 8, 8, 6, 2)  # r-chunk sizes (sum = R); tapered tail
KC = K // 128  # k chunks of 128 (PSUM partition limit)
WIN = 512 // PS  # r-rows per PSUM-bank-sized matmul window
FOLD_RC = 16  # max r-rows per PSUM fold tile (2 banks)
ALT_QUEUES = True  # alternate chunk DMAs between sync and scalar HWDGE queues

F32 = mybir.dt.float32
F16 = mybir.dt.float16


def build_bass() -> bass.Bass:
    nc = bacc.Bacc(None, target_bir_lowering=False)
    # per-core shards, fp16, p split into S slices: [d, s, r, p/S]
    ua = nc.declare_dram_parameter("ua", [DC, S, R, PS], F16, isOutput=False)
    ub = nc.declare_dram_parameter("ub", [DC, S, R, PS], F16, isOutput=False)
    # F/sqrt(D) with k on partitions: [128, KC, R] fp16
    f = nc.declare_dram_parameter("f", [128, KC, R], F16, isOutput=False)
    # (F/sqrt(D))^T: [R, K] fp16 (matmul lhsT)
    ft = nc.declare_dram_parameter("ft", [R, K], F16, isOutput=False)
    # out[p, c] = partial csi[c*128 + p]
    out = nc.declare_dram_parameter("out", [128, KC], F32, isOutput=True)

    with TileContext(nc) as tc:
        with (
            tc.tile_pool(name="const", bufs=1) as cpool,
            tc.tile_pool(name="chunks", bufs=6) as chpool,
            tc.tile_pool(name="small", bufs=1) as spool,
            tc.tile_pool(name="scratch", bufs=1) as scpool,
            tc.tile_pool(name="pfold", bufs=2, space="PSUM") as fpool,
            tc.tile_pool(name="pm", bufs=1, space="PSUM") as mpool,
            tc.tile_pool(name="pg", bufs=1, space="PSUM") as gpool,
        ):
            identity = cpool.tile([128, 128], F16)
            make_identity(nc, identity[:])

            u_a = spool.tile([DC, R], F16, tag="u_a")
            u_b = spool.tile([DC, R], F16, tag="u_b")

            with nc.allow_low_precision(reason="fp16 path is within tolerance"):
                # Streaming p-reduction: Ua[d,r] = sum_{s,p} ua[d,s,r,p]
                base = 0
                qi = 0
                for rc in CHUNKS:
                    for t_ap, u in ((ua, u_a), (ub, u_b)):
                        ch = chpool.tile([DC, S, rc, PS], F16, tag="chunk")
                        eng = nc.scalar if (ALT_QUEUES and qi % 2) else nc.sync
                        eng.dma_start(
                            out=ch[:], in_=t_ap[:, :, base : base + rc, :]
                        )
                        qi += 1
                        # PE fold over s: pf[d, fc, p] = sum_s ch[d, s, fc, p],
                        # one PSUM-bank window at a time; DVE reduces each
                        # window over p as soon as its fold completes.
                        for f0 in range(0, rc, FOLD_RC):
                            f1 = min(f0 + FOLD_RC, rc)
                            pf = fpool.tile([DC, f1 - f0, PS], F32, tag="fold")
                            for w0 in range(0, f1 - f0, WIN):
                                w1 = min(w0 + WIN, f1 - f0)
                                for s in range(S):
                                    nc.tensor.matmul(
                                        pf[:, w0:w1, :],
                                        identity[:],
                                        ch[:, s, f0 + w0 : f0 + w1, :],
                                        start=(s == 0),
                                        stop=(s == S - 1),
                                    )
                                nc.vector.tensor_reduce(
                                    out=u[:, base + f0 + w0 : base + f0 + w1],
                                    in_=pf[:, w0:w1, :],
                                    axis=mybir.AxisListType.X,
                                    op=mybir.AluOpType.add,
                                )
                    base += rc

                # F tiles (queued after the bulk stream; needed only at tail)
                f_sb = cpool.tile([128, KC, R], F16)
                nc.sync.dma_start(out=f_sb[:], in_=f[:])
                ft_sb = cpool.tile([R, K], F16)
                nc.scalar.dma_start(out=ft_sb[:], in_=ft[:])

                # Gram matrix M[r1,r2] = sum_d Ua[d,r1] Ub[d,r2]
                m_psum = mpool.tile([R, R], F32, tag="m")
                nc.tensor.matmul(m_psum[:], u_a[:], u_b[:], start=True, stop=True)
                m_sb = spool.tile([R, R], F16, tag="m_sb")
                nc.vector.tensor_copy(out=m_sb[:], in_=m_psum[:])

                # g[k, c, r2] = sum_r1 F'[k+128c, r1] M[r1, r2], all KC chunks
                # into ONE PSUM tile so the csi tail is one mul + one reduce.
                g_psum = gpool.tile([128, KC, R], F32, tag="g")
                for c in range(KC):
                    nc.tensor.matmul(
                        g_psum[:, c, :],
                        ft_sb[:, c * 128 : (c + 1) * 128],
                        m_sb[:],
                        start=True,
                        stop=True,
                    )
                scr = scpool.tile([128, KC, R], F32, tag="scr")
                nc.vector.tensor_mul(out=scr[:], in0=g_psum[:], in1=f_sb[:])
                csi = spool.tile([128, KC], F32, tag="csi")
                nc.vector.tensor_reduce(
                    out=csi[:],
                    in_=scr[:],
                    axis=mybir.AxisListType.X,
                    op=mybir.AluOpType.add,
                )
            nc.sync.dma_start(out=out[:], in_=csi[:])
    nc.compile()
    return nc


def _prep_inputs(ua, ub, f):
    """Host-side layout prep shared by kernel() and test harnesses."""
    # [D, P, R] fp32 -> [D, S, R, P/S] fp16 (p split outer for the PE fold)
    ua16 = ua.astype(np.float16).reshape(D, S, PS, R).transpose(0, 1, 3, 2)
    ub16 = ub.astype(np.float16).reshape(D, S, PS, R).transpose(0, 1, 3, 2)
    ua16 = np.ascontiguousarray(ua16)
    ub16 = np.ascontiguousarray(ub16)
    fs = (f / np.sqrt(np.float32(D))).astype(np.float32)
    f_host = np.ascontiguousarray(
        fs.reshape(KC, 128, R).transpose(1, 0, 2).astype(np.float16)
    )
    ft_host = np.ascontiguousarray(fs.T.astype(np.float16))
    return ua16, ub16, f_host, ft_host


_NC_CACHE = None


def kernel(**inputs: np.ndarray) -> np.ndarray:
    global _NC_CACHE
    ua = np.asarray(inputs["attenuation_vectors"], dtype=np.float32)
    ub = np.asarray(inputs["radiation_vectors"], dtype=np.float32)
    f = np.asarray(inputs["frequency_basis_vectors"], dtype=np.float32)

    ua16, ub16, f_host, ft_host = _prep_inputs(ua, ub, f)

    if _NC_CACHE is None:
        _NC_CACHE = build_bass()
    nc = _NC_CACHE

    in_maps = [
        {
            "ua": ua16[c * DC : (c + 1) * DC],
            "ub": ub16[c * DC : (c + 1) * DC],
            "f": f_host,
            "ft": ft_host,
        }
        for c in range(NCORES)
    ]
    res = run_bass_kernel_spmd(nc, in_maps, list(range(NCORES)))
    acc = np.zeros((128, KC), dtype=np.float32)
    for r in res.results:
        acc += r["out"]
    return acc.T.reshape(K).astype(np.float32)


if __name__ == "__main__":
    rng = np.random.default_rng(0)
    ins = {
        "attenuation_vectors": rng.standard_normal((D, P, R), dtype=np.float32),
        "radiation_vectors": rng.standard_normal((D, P, R), dtype=np.float32),
        "frequency_basis_vectors": rng.standard_normal((K, R), dtype=np.float32),
    }
    got = kernel(**ins)
    ua_s = ins["attenuation_vectors"].sum(axis=1)
    ub_s = ins["radiation_vectors"].sum(axis=1)
    a = ua_s @ ins["frequency_basis_vectors"].T
    b = ub_s @ ins["frequency_basis_vectors"].T
    want = (a * b).sum(axis=0) / D
    err = np.abs(got - want).max() / np.abs(want).max()
    print("rel err vs local numpy:", err)


# revision 23
# speedup vs baseline: 3.0415x; 1.0232x over previous
"""Low-rank ray tracer CSI kernel for 8 Trainium2 NeuronCores.

Reference computation:
    A = einsum('dpr,kr->dk', ua, F); B = einsum('dpr,kr->dk', ub, F)
    csi[k] = sum_d A[d,k]*B[d,k] / D

Since F has no p index, A = (sum_p ua) @ F^T.  Let Ua[d,r] = sum_p ua[d,p,r]
(same for Ub).  Then
    csi[k] = (1/D) * f_k^T (Ua^T Ub) f_k  =  f'_k^T M f'_k
with M = Ua^T Ub a tiny [R,R] Gram matrix and f' = f/sqrt(D) (scaling folded
into F on the host).  Sharding d across cores makes M additive and csi linear
in M, so each core returns its partial csi and the host sums 8 vectors of 4KB.

The kernel is DMA-bound: each core streams its shard once.  To halve HBM
traffic the host casts the inputs to fp16 (the 2e-2 rel-err budget dwarfs
fp16's ~5e-4).  The p-reduction is split between engines so neither blocks
the DMA stream: the host pre-splits p into S=4 slices (layout [D, S, R, P/S]),
the otherwise-idle PE folds the slices with accumulating identity matmuls
(PSUM += I @ slice), and the DVE only reduces the remaining P/S=64-wide axis.
The r-chunks are tapered (8,...,8,6,2) so the final fold+reduce on the
critical path after the last DMA is small; 8-row chunks keep DMA descriptors
at 1KB, which still saturates the ~358 GB/s per-core HBM limit (the SDMA
fabric overhead at 1KB descriptors stays above the HBM ceiling).
"""

import sys

import numpy as np

sys.path.insert(0, "/opt/trn_rl_repo")

import concourse.bacc as bacc
import concourse.bass as bass
import concourse.mybir as mybir
from concourse.bass_utils import run_bass_kernel_spmd
from concourse.masks import make_identity
from concourse.tile import TileContext

D, P, R, K = 1024, 256, 64, 1024
NCORES = 8
DC = D // NCORES  # directions per core
S = 4  # p-slices folded on the PE
PS = P // S  # p per slice after the fold
UA_CHUNKS = (8, 8, 8, 8, 8, 8, 8, 8)  # r-chunk sizes for ua (streamed first)
UB_CHUNKS = (8, 8, 8, 8, 8, 8, 8, 6, 2)  # for ub; tapered tail
BLOCK_SPLIT = 56  # r2 boundary: csi cols 0:56 computed mid-stream, 56:64 in tail
KC = K // 128  # k chunks of 128 (PSUM partition limit)
WIN = 512 // PS  # r-rows per PSUM-bank-sized matmul window
FOLD_RC = 16  # max r-rows per PSUM fold tile (2 banks)
ALT_QUEUES = True  # alternate chunk DMAs between sync and scalar HWDGE queues

F32 = mybir.dt.float32
F16 = mybir.dt.float16


def build_bass() -> bass.Bass:
    nc = bacc.Bacc(None, target_bir_lowering=False)
    # per-core shards, fp16, p split into S slices: [d, s, r, p/S]
    ua = nc.declare_dram_parameter("ua", [DC, S, R, PS], F16, isOutput=False)
    ub = nc.declare_dram_parameter("ub", [DC, S, R, PS], F16, isOutput=False)
    # F/sqrt(D) with k on partitions: [128, KC, R] fp16
    f = nc.declare_dram_parameter("f", [128, KC, R], F16, isOutput=False)
    # (F/sqrt(D))^T: [R, K] fp16 (matmul lhsT)
    ft = nc.declare_dram_parameter("ft", [R, K], F16, isOutput=False)
    # out[p, c] = partial csi[c*128 + p]
    out = nc.declare_dram_parameter("out", [128, KC], F32, isOutput=True)

    with TileContext(nc) as tc:
        with (
            tc.tile_pool(name="const", bufs=1) as cpool,
            tc.tile_pool(name="chunks", bufs=6) as chpool,
            tc.tile_pool(name="small", bufs=1) as spool,
            tc.tile_pool(name="scratch", bufs=1) as scpool,
            tc.tile_pool(name="pfold", bufs=2, space="PSUM") as fpool,
            tc.tile_pool(name="pm", bufs=1, space="PSUM") as mpool,
            tc.tile_pool(name="pg", bufs=1, space="PSUM") as gpool,
        ):
            identity = cpool.tile([128, 128], F16)
            make_identity(nc, identity[:])

            u_a = spool.tile([DC, R], F16, tag="u_a")
            u_b = spool.tile([DC, R], F16, tag="u_b")

            qi = 0

            def stream_chunk(t_ap, u, base, rc):
                """DMA one [*, S, rc, PS] chunk, PE-fold s, DVE-reduce p."""
                nonlocal qi
                ch = chpool.tile([DC, S, rc, PS], F16, tag="chunk")
                eng = nc.scalar if (ALT_QUEUES and qi % 2) else nc.sync
                eng.dma_start(out=ch[:], in_=t_ap[:, :, base : base + rc, :])
                qi += 1
                for f0 in range(0, rc, FOLD_RC):
                    f1 = min(f0 + FOLD_RC, rc)
                    pf = fpool.tile([DC, f1 - f0, PS], F32, tag="fold")
                    for w0 in range(0, f1 - f0, WIN):
                        w1 = min(w0 + WIN, f1 - f0)
                        for s in range(S):
                            nc.tensor.matmul(
                                pf[:, w0:w1, :],
                                identity[:],
                                ch[:, s, f0 + w0 : f0 + w1, :],
                                start=(s == 0),
                                stop=(s == S - 1),
                            )
                        nc.vector.tensor_reduce(
                            out=u[:, base + f0 + w0 : base + f0 + w1],
                            in_=pf[:, w0:w1, :],
                            axis=mybir.AxisListType.X,
                            op=mybir.AluOpType.add,
                        )

            with nc.allow_low_precision(reason="fp16 path is within tolerance"):
                # Phase 1: stream all of ua; Ua[d,r] = sum_{s,p} ua[d,s,r,p]
                base = 0
                for rc in UA_CHUNKS:
                    stream_chunk(ua, u_a, base, rc)
                    base += rc

                # F tiles (mid-stream; needed when the first M block lands)
                f_sb = cpool.tile([128, KC, R], F16)
                nc.sync.dma_start(out=f_sb[:], in_=f[:])
                ft_sb = cpool.tile([R, K], F16)
                nc.scalar.dma_start(out=ft_sb[:], in_=ft[:])

                # Phase 2: stream ub; emit the csi work for r2 block 0:56
                # as soon as u_b's first 56 columns are reduced, so only the
                # last 8 columns' (tiny) csi work sits after the stream.
                m_psum = mpool.tile([R, R], F32, tag="m")
                m_sb = spool.tile([R, R], F16, tag="m_sb")
                g_psum = gpool.tile([128, KC, R], F32, tag="g")
                csi_parts = []

                def csi_block(b0, b1):
                    """csi_part[k] = sum_{r2 in [b0,b1)} g[k,r2] * F'[k,r2]."""
                    nc.tensor.matmul(
                        m_psum[:, b0:b1], u_a[:], u_b[:, b0:b1], start=True, stop=True
                    )
                    nc.vector.tensor_copy(out=m_sb[:, b0:b1], in_=m_psum[:, b0:b1])
                    for c in range(KC):
                        nc.tensor.matmul(
                            g_psum[:, c, b0:b1],
                            ft_sb[:, c * 128 : (c + 1) * 128],
                            m_sb[:, b0:b1],
                            start=True,
                            stop=True,
                        )
                    scr = scpool.tile([128, KC, b1 - b0], F32, tag=f"scr{b0}")
                    nc.vector.tensor_mul(
                        out=scr[:], in0=g_psum[:, :, b0:b1], in1=f_sb[:, :, b0:b1]
                    )
                    part = spool.tile([128, KC], F32, tag=f"csi{b0}")
                    nc.vector.tensor_reduce(
                        out=part[:],
                        in_=scr[:],
                        axis=mybir.AxisListType.X,
                        op=mybir.AluOpType.add,
                    )
                    csi_parts.append(part)

                base = 0
                for rc in UB_CHUNKS:
                    stream_chunk(ub, u_b, base, rc)
                    base += rc
                    if base == BLOCK_SPLIT:
                        csi_block(0, BLOCK_SPLIT)
                csi_block(BLOCK_SPLIT, R)

                csi = spool.tile([128, KC], F32, tag="csi")
                nc.vector.tensor_add(
                    out=csi[:], in0=csi_parts[0][:], in1=csi_parts[1][:]
                )
            nc.sync.dma_start(out=out[:], in_=csi[:])
    nc.compile()
    return nc


def _prep_inputs(ua, ub, f):
    """Host-side layout prep shared by kernel() and test harnesses."""
    # [D, P, R] fp32 -> [D, S, R, P/S] fp16 (p split outer for the PE fold)
    ua16 = ua.astype(np.float16).reshape(D, S, PS, R).transpose(0, 1, 3, 2)
    ub16 = ub.astype(np.float16).reshape(D, S, PS, R).transpose(0, 1, 3, 2)
    ua16 = np.ascontiguousarray(ua16)
    ub16 = np.ascontiguousarray(ub16)
    fs = (f / np.sqrt(np.float32(D))).astype(np.float32)
    f_host = np.ascontiguousarray(
        fs.reshape(KC, 128, R).transpose(1, 0, 2).astype(np.float16)
    )
    ft_host = np.ascontiguousarray(fs.T.astype(np.float16))
    return ua16, ub16, f_host, ft_host


_NC_CACHE = None


def kernel(**inputs: np.ndarray) -> np.ndarray:
    global _NC_CACHE
    ua = np.asarray(inputs["attenuation_vectors"], dtype=np.float32)
    ub = np.asarray(inputs["radiation_vectors"], dtype=np.float32)
    f = np.asarray(inputs["frequency_basis_vectors"], dtype=np.float32)

    ua16, ub16, f_host, ft_host = _prep_inputs(ua, ub, f)

    if _NC_CACHE is None:
        _NC_CACHE = build_bass()
    nc = _NC_CACHE

    in_maps = [
        {
            "ua": ua16[c * DC : (c + 1) * DC],
            "ub": ub16[c * DC : (c + 1) * DC],
            "f": f_host,
            "ft": ft_host,
        }
        for c in range(NCORES)
    ]
    res = run_bass_kernel_spmd(nc, in_maps, list(range(NCORES)))
    acc = np.zeros((128, KC), dtype=np.float32)
    for r in res.results:
        acc += r["out"]
    return acc.T.reshape(K).astype(np.float32)


if __name__ == "__main__":
    rng = np.random.default_rng(0)
    ins = {
        "attenuation_vectors": rng.standard_normal((D, P, R), dtype=np.float32),
        "radiation_vectors": rng.standard_normal((D, P, R), dtype=np.float32),
        "frequency_basis_vectors": rng.standard_normal((K, R), dtype=np.float32),
    }
    got = kernel(**ins)
    ua_s = ins["attenuation_vectors"].sum(axis=1)
    ub_s = ins["radiation_vectors"].sum(axis=1)
    a = ua_s @ ins["frequency_basis_vectors"].T
    b = ub_s @ ins["frequency_basis_vectors"].T
    want = (a * b).sum(axis=0) / D
    err = np.abs(got - want).max() / np.abs(want).max()
    print("rel err vs local numpy:", err)
